# revision 1
# baseline (speedup 1.0000x reference)
"""Two-layer GCN encoder (GCNConv x2 + minmax + L2 normalize) on 8 TRN2 NeuronCores.

Sharding: nodes row-partitioned across 8 cores (12500/core); each edge owned by the
core owning its destination. Per core, edges are grouped by 128-node destination
block and by source chunk (25000-row table chunks keep dma_gather's int16 indices
in range), padded to 128-edge tiles; per-(block,chunk) tile counts are equalized
across cores so one SPMD program serves all 8.

Per layer: the (N x d) linear-transform table is computed shard-wise, AllGathered,
then each superblock of 4 destination blocks issues one dma_gather per source
chunk (128-row batches of 512B/256B rows). Aggregation on TensorE: per 128-edge
tile a selection matrix S[e,j] = norm[e] * (dst_local[e]==j) is built with one
fused DVE tensor_scalar (is_equal then mult vs an iota row); layer 1 accumulates
Msg^T @ S (transposed, so the +b1 bias and the h@W2 lhsT need no transpose),
layer 2 accumulates S^T @ Msg node-major, followed by minmax + L2 normalize.
"""

import math

import numpy as np

import concourse.bass as bass
import concourse.bacc as bacc
import concourse.mybir as mybir
import concourse.tile as tile
from concourse import bass_utils

NCORES = 8
BLK = 128
IN_C = 128
HID = 128
OUT_C = 64
CHUNK_ROWS = 25000  # dma_gather idx is int16: chunk-relative indices < 32768
SBN = 4  # destination blocks per gather superblock

LAST_RESULTS = None
_PROGRAM_CACHE = {}


def _host_prep(x, edge_index):
    n = x.shape[0]
    assert n % NCORES == 0
    npc = n // NCORES
    nblk = math.ceil(npc / BLK)
    n_chunks = math.ceil(n / CHUNK_ROWS)

    src = edge_index[0].astype(np.int64)
    dst = edge_index[1].astype(np.int64)

    deg = (np.bincount(dst, minlength=n) + 1).astype(np.float32)
    dinv = (1.0 / np.sqrt(deg)).astype(np.float32)

    loop = np.arange(n, dtype=np.int64)
    s_all = np.concatenate([src, loop])
    d_all = np.concatenate([dst, loop])
    norm_all = (dinv[s_all] * dinv[d_all]).astype(np.float32)

    core = d_all // npc
    within = d_all % npc
    blk = within // BLK
    colv = (within % BLK).astype(np.float32)
    chunk = s_all // CHUNK_ROWS

    key = (core * nblk + blk) * n_chunks + chunk
    counts = np.bincount(key, minlength=NCORES * nblk * n_chunks).reshape(
        NCORES, nblk * n_chunks
    )
    # tiles per (block, chunk), equalized across cores (SPMD)
    t4 = ((counts + BLK - 1) // BLK).max(axis=0).reshape(nblk, n_chunks)

    # global tile order: for each superblock: for each chunk: for each block: tiles
    gofs = np.zeros((nblk, n_chunks), np.int64)
    cur = 0
    n_sb = math.ceil(nblk / SBN)
    for sbi in range(n_sb):
        for q in range(n_chunks):
            for b in range(sbi * SBN, min((sbi + 1) * SBN, nblk)):
                gofs[b, q] = cur
                cur += int(t4[b, q])
    t_total = cur

    order = np.argsort(key, kind="stable")
    ks = key[order]
    ss = s_all[order]
    cs = colv[order]
    nn = norm_all[order]

    group_start = np.zeros(NCORES * nblk * n_chunks, np.int64)
    group_start[1:] = np.cumsum(counts.ravel())[:-1]
    r = np.arange(len(ks), dtype=np.int64) - group_start[ks]
    t_idx = r // BLK
    p_idx = r % BLK
    c_idx = ks // (nblk * n_chunks)
    b_idx = (ks // n_chunks) % nblk
    q_idx = ks % n_chunks
    gcol = gofs[b_idx, q_idx] + t_idx
    rel = (ss - q_idx * CHUNK_ROWS).astype(np.int16)

    # int16 idx stream for dma_gather: index k of a call lives at
    # [k%16 (+16*replica), call_col0*8 + k//16]; with 128-multiple groups this
    # reduces to [p%16, gcol*8 + p//16] independent of call boundaries.
    srcs16 = np.zeros((NCORES, 16, t_total * 8), np.int16)
    dstf_arr = np.zeros((NCORES, BLK, t_total), np.float32)
    normf_arr = np.zeros((NCORES, BLK, t_total), np.float32)
    srcs16[c_idx, p_idx % 16, gcol * 8 + p_idx // 16] = rel
    dstf_arr[c_idx, p_idx, gcol] = cs
    normf_arr[c_idx, p_idx, gcol] = nn
    srcs16 = np.tile(srcs16, (1, 8, 1))  # replicate for the 8 Q7 cores

    xt = np.ascontiguousarray(x.T.astype(np.float32))
    in_maps = []
    for c in range(NCORES):
        in_maps.append(
            {
                "xT": np.ascontiguousarray(xt[:, c * npc : (c + 1) * npc]),
                "srcs16": np.ascontiguousarray(srcs16[c]),
                "dstf": np.ascontiguousarray(dstf_arr[c]),
                "normf": np.ascontiguousarray(normf_arr[c]),
            }
        )
    return in_maps, t4, gofs, npc, nblk, n_chunks


def _build_nc(n, npc, nblk, n_chunks, t4, gofs):
    t_total = int(t4.sum())
    f32 = mybir.dt.float32
    i16 = mybir.dt.int16
    n_sb = math.ceil(nblk / SBN)

    nc = bacc.Bacc(
        "TRN2",
        target_bir_lowering=False,
        debug=False,
        enable_asserts=False,
        num_devices=NCORES,
    )

    xT = nc.dram_tensor("xT", [IN_C, npc], f32, kind="ExternalInput").ap()
    W1 = nc.dram_tensor("W1", [IN_C, HID], f32, kind="ExternalInput").ap()
    W2 = nc.dram_tensor("W2", [HID, OUT_C], f32, kind="ExternalInput").ap()
    b1c = nc.dram_tensor("b1c", [HID, 1], f32, kind="ExternalInput").ap()
    b2b = nc.dram_tensor("b2b", [BLK, OUT_C], f32, kind="ExternalInput").ap()
    iota = nc.dram_tensor("iota", [BLK, BLK], f32, kind="ExternalInput").ap()
    srcs16 = nc.dram_tensor(
        "srcs16", [BLK, t_total * 8], i16, kind="ExternalInput"
    ).ap()
    dstf = nc.dram_tensor("dstf", [BLK, t_total], f32, kind="ExternalInput").ap()
    normf = nc.dram_tensor("normf", [BLK, t_total], f32, kind="ExternalInput").ap()
    out = nc.dram_tensor("out", [npc, OUT_C], f32, kind="ExternalOutput").ap()

    ieq = mybir.AluOpType.is_equal
    mul = mybir.AluOpType.mult
    sub = mybir.AluOpType.subtract

    def nb_of(b):
        return min(BLK, npc - b * BLK)

    def sb_blocks(sbi):
        return range(sbi * SBN, min((sbi + 1) * SBN, nblk))

    with tile.TileContext(nc) as tc:
        with (
            tc.tile_pool(name="dram", bufs=1, space="DRAM") as dram,
            tc.tile_pool(name="const", bufs=1) as constp,
            tc.tile_pool(name="meta", bufs=1) as metap,
            tc.tile_pool(name="io", bufs=3) as iop,
            tc.tile_pool(name="idx", bufs=2) as idxp,
            tc.tile_pool(name="msg", bufs=2) as msgp,
            tc.tile_pool(name="sel", bufs=4) as selp,
            tc.tile_pool(name="fin", bufs=2) as finp,
            tc.tile_pool(name="stat", bufs=3) as statp,
            tc.tile_pool(name="psA", bufs=2, space="PSUM") as psA,
            tc.tile_pool(name="psB", bufs=2, space="PSUM") as psB,
        ):
            xw1_shard = dram.tile([npc, HID], f32)
            xw1_full = dram.tile([n, HID], f32, addr_space="Shared")
            hw2_shard = dram.tile([npc, OUT_C], f32)
            hw2_full = dram.tile([n, OUT_C], f32, addr_space="Shared")

            W1s = constp.tile([IN_C, HID], f32)
            nc.sync.dma_start(out=W1s[:], in_=W1)
            W2s = constp.tile([HID, OUT_C], f32)
            nc.sync.dma_start(out=W2s[:], in_=W2)
            b1s = constp.tile([HID, 1], f32)
            nc.sync.dma_start(out=b1s[:], in_=b1c)
            b2s = constp.tile([BLK, OUT_C], f32)
            nc.sync.dma_start(out=b2s[:], in_=b2b)
            iotas = constp.tile([BLK, BLK], f32)
            nc.sync.dma_start(out=iotas[:], in_=iota)
            dstf_s = metap.tile([BLK, t_total], f32)
            nc.sync.dma_start(out=dstf_s[:], in_=dstf)
            normf_s = metap.tile([BLK, t_total], f32)
            nc.sync.dma_start(out=normf_s[:], in_=normf)

            # ---- Phase 0: xw1_shard = x_c @ W1 ----
            XCH = 4
            for bc in range(0, nblk, XCH):
                hi = min(bc + XCH, nblk)
                w = min(hi * BLK, npc) - bc * BLK
                xt_t = iop.tile([IN_C, XCH * BLK], f32, tag="xt")
                nc.sync.dma_start(out=xt_t[:, :w], in_=xT[:, bc * BLK : bc * BLK + w])
                for b in range(bc, hi):
                    nb = nb_of(b)
                    o = (b - bc) * BLK
                    ps = psA.tile([BLK, HID], f32, tag="psA")
                    nc.tensor.matmul(
                        out=ps[:nb, :],
                        lhsT=xt_t[:, o : o + nb],
                        rhs=W1s[:],
                        start=True,
                        stop=True,
                    )
                    xw_t = iop.tile([BLK, HID], f32, tag="xw")
                    nc.scalar.copy(xw_t[:nb, :], ps[:nb, :])
                    nc.sync.dma_start(
                        out=xw1_shard[b * BLK : b * BLK + nb, :], in_=xw_t[:nb, :]
                    )

            nc.gpsimd.collective_compute(
                "AllGather",
                mybir.AluOpType.bypass,
                replica_groups=[list(range(NCORES))],
                ins=[xw1_shard[:]],
                outs=[xw1_full[:]],
            )

            def gather_sb(sbi, table_full, elem, msg_tag):
                """One superblock's gathers: returns (msg tile, sb_col0, T_sb)."""
                blocks = list(sb_blocks(sbi))
                sb_col0 = int(gofs[blocks[0], 0])
                t_sb = int(sum(t4[b, q] for b in blocks for q in range(n_chunks)))
                idx_t = idxp.tile([BLK, t_sb * 8], i16, tag="idx")
                nc.sync.dma_start(
                    out=idx_t[:],
                    in_=srcs16[:, sb_col0 * 8 : (sb_col0 + t_sb) * 8],
                )
                msg = msgp.tile([BLK, t_sb * elem], f32, tag=msg_tag)
                MAXT = 8  # dma_gather caps at 1024 indices per call
                for q in range(n_chunks):
                    cs = int(gofs[blocks[0], q])
                    tq = int(sum(t4[b, q] for b in blocks))
                    hi_r = min((q + 1) * CHUNK_ROWS, n)
                    for k in range(0, tq, MAXT):
                        tk = min(MAXT, tq - k)
                        lo = cs - sb_col0 + k
                        nc.gpsimd.dma_gather(
                            out_ap=msg[:, lo * elem : (lo + tk) * elem].rearrange(
                                "p (t e) -> p t e", e=elem
                            ),
                            in_ap=table_full[q * CHUNK_ROWS : hi_r, :],
                            idxs_ap=idx_t[:, lo * 8 : (lo + tk) * 8],
                            num_idxs=tk * BLK,
                            num_idxs_reg=tk * BLK,
                            elem_size=elem,
                        )
                return msg, sb_col0, t_sb

            def block_tiles(b):
                return [
                    int(gofs[b, q] + t)
                    for q in range(n_chunks)
                    for t in range(int(t4[b, q]))
                ]

            # ---- Phase 1+2 ----
            for sbi in range(n_sb):
                msg, sb_col0, _ = gather_sb(sbi, xw1_full, HID, "msg")
                for b in sb_blocks(sbi):
                    nb = nb_of(b)
                    cols = block_tiles(b)
                    psT = psA.tile([BLK, HID], f32, tag="psA")
                    for j, g in enumerate(cols):
                        S = selp.tile([BLK, BLK], f32, tag="S")
                        nc.vector.tensor_scalar(
                            out=S[:],
                            in0=iotas[:],
                            scalar1=dstf_s[:, g : g + 1],
                            scalar2=normf_s[:, g : g + 1],
                            op0=ieq,
                            op1=mul,
                        )
                        lo = g - sb_col0
                        nc.tensor.matmul(
                            out=psT[:, :nb],
                            lhsT=msg[:, lo * HID : (lo + 1) * HID],
                            rhs=S[:, :nb],
                            start=(j == 0),
                            stop=(j == len(cols) - 1),
                        )
                    hT = finp.tile([HID, BLK], f32, tag="hT")
                    nc.vector.tensor_scalar_add(hT[:, :nb], psT[:, :nb], b1s[:])
                    ps2 = psB.tile([BLK, OUT_C], f32, tag="psB")
                    nc.tensor.matmul(
                        out=ps2[:nb, :],
                        lhsT=hT[:, :nb],
                        rhs=W2s[:],
                        start=True,
                        stop=True,
                    )
                    hw2_t = finp.tile([BLK, OUT_C], f32, tag="hw2")
                    nc.scalar.copy(hw2_t[:nb, :], ps2[:nb, :])
                    nc.sync.dma_start(
                        out=hw2_shard[b * BLK : b * BLK + nb, :], in_=hw2_t[:nb, :]
                    )

            nc.gpsimd.collective_compute(
                "AllGather",
                mybir.AluOpType.bypass,
                replica_groups=[list(range(NCORES))],
                ins=[hw2_shard[:]],
                outs=[hw2_full[:]],
            )

            # ---- Phase 3 ----
            for sbi in range(n_sb):
                msg2, sb_col0, _ = gather_sb(sbi, hw2_full, OUT_C, "msg")
                for b in sb_blocks(sbi):
                    nb = nb_of(b)
                    cols = block_tiles(b)
                    psO = psB.tile([BLK, OUT_C], f32, tag="psB")
                    for j, g in enumerate(cols):
                        S = selp.tile([BLK, BLK], f32, tag="S")
                        nc.vector.tensor_scalar(
                            out=S[:],
                            in0=iotas[:],
                            scalar1=dstf_s[:, g : g + 1],
                            scalar2=normf_s[:, g : g + 1],
                            op0=ieq,
                            op1=mul,
                        )
                        lo = g - sb_col0
                        nc.tensor.matmul(
                            out=psO[:nb, :],
                            lhsT=S[:, :nb],
                            rhs=msg2[:, lo * OUT_C : (lo + 1) * OUT_C],
                            start=(j == 0),
                            stop=(j == len(cols) - 1),
                        )
                    z = finp.tile([BLK, OUT_C], f32, tag="z")
                    nc.vector.tensor_add(z[:nb, :], psO[:nb, :], b2s[:nb, :])
                    zmax = statp.tile([BLK, 1], f32, tag="zmax")
                    nc.vector.tensor_reduce(
                        zmax[:nb], z[:nb, :], axis=mybir.AxisListType.X,
                        op=mybir.AluOpType.max,
                    )
                    zmin = statp.tile([BLK, 1], f32, tag="zmin")
                    nc.vector.tensor_reduce(
                        zmin[:nb], z[:nb, :], axis=mybir.AxisListType.X,
                        op=mybir.AluOpType.min,
                    )
                    rng_t = statp.tile([BLK, 1], f32, tag="rng")
                    nc.vector.tensor_sub(rng_t[:nb], zmax[:nb], zmin[:nb])
                    rinv = statp.tile([BLK, 1], f32, tag="rinv")
                    nc.vector.reciprocal(rinv[:nb], rng_t[:nb])
                    zs = finp.tile([BLK, OUT_C], f32, tag="zs")
                    nc.vector.tensor_scalar(
                        out=zs[:nb, :],
                        in0=z[:nb, :],
                        scalar1=zmin[:nb],
                        scalar2=rinv[:nb],
                        op0=sub,
                        op1=mul,
                    )
                    sq = finp.tile([BLK, OUT_C], f32, tag="sq")
                    ssq = statp.tile([BLK, 1], f32, tag="ssq")
                    nc.scalar.activation(
                        sq[:nb, :],
                        zs[:nb, :],
                        mybir.ActivationFunctionType.Square,
                        accum_out=ssq[:nb],
                    )
                    snrm = statp.tile([BLK, 1], f32, tag="snrm")
                    nc.scalar.sqrt(snrm[:nb], ssq[:nb])
                    nc.vector.tensor_scalar_max(snrm[:nb], snrm[:nb], 1e-12)
                    ninv = statp.tile([BLK, 1], f32, tag="ninv")
                    nc.vector.reciprocal(ninv[:nb], snrm[:nb])
                    res = finp.tile([BLK, OUT_C], f32, tag="res")
                    nc.vector.tensor_scalar_mul(res[:nb, :], zs[:nb, :], ninv[:nb])
                    nc.sync.dma_start(
                        out=out[b * BLK : b * BLK + nb, :], in_=res[:nb, :]
                    )

    nc.compile()
    return nc


def kernel(x, edge_index, W1, b1, W2, b2, trace=False):
    global LAST_RESULTS
    x = np.asarray(x)
    edge_index = np.asarray(edge_index)
    W1 = np.asarray(W1, dtype=np.float32)
    b1 = np.asarray(b1, dtype=np.float32)
    W2 = np.asarray(W2, dtype=np.float32)
    b2 = np.asarray(b2, dtype=np.float32)

    n = x.shape[0]
    in_maps, t4, gofs, npc, nblk, n_chunks = _host_prep(x, edge_index)

    consts = {
        "W1": np.ascontiguousarray(W1),
        "W2": np.ascontiguousarray(W2),
        "b1c": np.ascontiguousarray(b1.reshape(HID, 1)),
        "b2b": np.ascontiguousarray(np.tile(b2.reshape(1, OUT_C), (BLK, 1))),
        "iota": np.tile(np.arange(BLK, dtype=np.float32), (BLK, 1)),
    }
    for m in in_maps:
        m.update(consts)

    key = (n, t4.tobytes())
    nc = _PROGRAM_CACHE.get(key)
    if nc is None:
        nc = _build_nc(n, npc, nblk, n_chunks, t4, gofs)
        _PROGRAM_CACHE[key] = nc

    results = bass_utils.run_bass_kernel_spmd(
        nc, in_maps, core_ids=list(range(NCORES)), trace=trace
    )
    LAST_RESULTS = results
    return np.concatenate([results.results[c]["out"] for c in range(NCORES)], axis=0)



# revision 7
# speedup vs baseline: 1.0354x; 1.0354x over previous
"""Two-layer GCN encoder (GCNConv x2 + minmax + L2 normalize) on 8 TRN2 NeuronCores.

Algebra: with A = D^-1/2 (Adj+I) D^-1/2 and no nonlinearity between the two
GCNConv layers, out = minmax_l2( A.(A.x).(W1@W2) + rowsumA.(b1@W2) + b2 ).
The symmetric norm factorizes: each aggregation is dinv[d] * sum_e t[src_e]
with the table pre-scaled by dinv (t1 = dinv*x on host, t2 = dinv^2*(A-sum)
on device), so the per-tile selection matrix S[e,j] = (dst_local[e]==j) is
0/1 and built with a single is_equal DVE op in bf16.

Sharding: nodes row-partitioned across 8 cores (12500/core); each edge owned
by the core owning its destination. Edges grouped by 128-node destination
block and 25000-row source chunk (dma_gather int16 indices), padded to
128-edge tiles; per-(block,chunk) tile counts are equalized across cores so
one SPMD program serves all 8. Gathers issue one dma_gather per (superblock
of 4 blocks, chunk) (~23 tiles = ~2900 indices per call); trailing pad slots
carry -1 indices which the Q7 gather kernel trims at runtime.

Phase A aggregates t1 node-major per block (psum += S^T @ Msg), scales by
dinv^2 on ScalarE, stores bf16 shard, AllGathers to the full t2 table.
Phase B aggregates t2 transposed (psum += Msg^T @ S), applies W12 via a
second matmul, then dinv scale (ScalarE) + bias + minmax + L2 normalize.
"""

import math

import numpy as np
import ml_dtypes

import concourse.bass as bass
import concourse.bacc as bacc
import concourse.mybir as mybir
import concourse.tile as tile
from concourse import bass_utils

NCORES = 8
BLK = 128
IN_C = 128
HID = 128
OUT_C = 64
CHUNK_ROWS = 25000  # dma_gather idx is int16: chunk-relative indices < 32768
SBN = 4  # destination blocks per gather superblock
MAXT = 8  # tiles per dma_gather call (1024 idx)

BF16 = ml_dtypes.bfloat16

LAST_RESULTS = None
_PROGRAM_CACHE = {}


def _host_prep(x, edge_index, W1, b1, W2, b2):
    n = x.shape[0]
    assert n % NCORES == 0
    npc = n // NCORES
    nblk = math.ceil(npc / BLK)
    n_chunks = math.ceil(n / CHUNK_ROWS)

    src = edge_index[0].astype(np.int64)
    dst = edge_index[1].astype(np.int64)

    deg = (np.bincount(dst, minlength=n) + 1).astype(np.float32)
    dinv = (1.0 / np.sqrt(deg)).astype(np.float32)

    loop = np.arange(n, dtype=np.int64)
    s_all = np.concatenate([src, loop])
    d_all = np.concatenate([dst, loop])

    # rowsumA[d] = dinv[d] * sum_{e->d} dinv[src_e]  (self-loop included)
    acc = np.zeros(n, np.float32)
    np.add.at(acc, d_all, dinv[s_all])
    rowsumA = dinv * acc

    W12 = (W1.astype(np.float64) @ W2.astype(np.float64)).astype(np.float32)
    b1W2 = (b1 @ W2).astype(np.float32)
    bias_full = (rowsumA[:, None] * b1W2[None, :] + b2[None, :]).astype(np.float32)

    t1 = (x * dinv[:, None]).astype(BF16)

    core = d_all // npc
    within = d_all % npc
    blk = within // BLK
    colv = (within % BLK).astype(np.float32)
    chunk = s_all // CHUNK_ROWS

    key = (core * nblk + blk) * n_chunks + chunk
    counts = np.bincount(key, minlength=NCORES * nblk * n_chunks).reshape(
        NCORES, nblk * n_chunks
    )
    # tiles per (block, chunk), equalized across cores (SPMD)
    t4 = ((counts + BLK - 1) // BLK).max(axis=0).reshape(nblk, n_chunks)

    # global tile order: for each superblock: for each chunk: for each block
    gofs = np.zeros((nblk, n_chunks), np.int64)
    cur = 0
    n_sb = math.ceil(nblk / SBN)
    for sbi in range(n_sb):
        for q in range(n_chunks):
            for b in range(sbi * SBN, min((sbi + 1) * SBN, nblk)):
                gofs[b, q] = cur
                cur += int(t4[b, q])
    t_total = cur

    order = np.argsort(key, kind="stable")
    ks = key[order]
    ss = s_all[order]
    cs = colv[order]

    group_start = np.zeros(NCORES * nblk * n_chunks, np.int64)
    group_start[1:] = np.cumsum(counts.ravel())[:-1]
    r = np.arange(len(ks), dtype=np.int64) - group_start[ks]
    t_idx = r // BLK
    p_idx = r % BLK
    c_idx = ks // (nblk * n_chunks)
    b_idx = (ks // n_chunks) % nblk
    q_idx = ks % n_chunks
    gcol = gofs[b_idx, q_idx] + t_idx
    rel = (ss - q_idx * CHUNK_ROWS).astype(np.int16)

    # int16 idx stream for dma_gather: index k of a call lives at
    # [k%16 (+16*replica), call_col0*8 + k//16]; with 128-multiple groups this
    # reduces to [p%16, gcol*8 + p//16] independent of call boundaries.
    srcs16 = np.zeros((NCORES, 16, t_total * 8), np.int16)
    dstf_arr = np.full((NCORES, BLK, t_total), -1.0, np.float32)
    srcs16[c_idx, p_idx % 16, gcol * 8 + p_idx // 16] = rel
    dstf_arr[c_idx, p_idx, gcol] = cs

    # Pad slots keep idx 0 (safe in-bounds read; their dstf=-1 zeroes S).
    # NOTE: -1 trailing-trim is unusable under SPMD — the Q7 kernel trims by
    # value but the sequencer reserves ring space by num_idxs_reg, which is a
    # compile-time constant shared across cores; a mismatch drifts the ring
    # offsets against the SDMA tail and executes stale descriptors.
    srcs16 = np.tile(srcs16, (1, 8, 1))  # replicate for the 8 Q7 cores

    # per-node dinv / dinv^2 by (block-local row, block); bias by block
    pad_npc = nblk * BLK
    dinv_pad = np.zeros((NCORES, pad_npc), np.float32)
    dinv_pad[:, :npc] = dinv.reshape(NCORES, npc)
    dinvB = np.ascontiguousarray(
        dinv_pad.reshape(NCORES, nblk, BLK).transpose(0, 2, 1)
    )
    bias_pad = np.zeros((NCORES, pad_npc, OUT_C), np.float32)
    bias_pad[:, :npc] = bias_full.reshape(NCORES, npc, OUT_C)
    biasB = np.ascontiguousarray(
        bias_pad.reshape(NCORES, nblk, BLK, OUT_C).transpose(0, 2, 1, 3)
    ).reshape(NCORES, BLK, nblk * OUT_C)

    iota = np.tile(np.arange(BLK, dtype=np.float32), (BLK, 1)).astype(BF16)

    in_maps = []
    for c in range(NCORES):
        in_maps.append(
            {
                "t1": t1,
                "srcs16": np.ascontiguousarray(srcs16[c]),
                "dstf": np.ascontiguousarray(dstf_arr[c]),
                "dinvB": np.ascontiguousarray(dinvB[c]),
                "dinvB2": np.ascontiguousarray(dinvB[c] ** 2),
                "biasB": np.ascontiguousarray(biasB[c]),
                "iota": iota,
                "W12": np.ascontiguousarray(W12),
            }
        )
    return in_maps, t4, gofs, npc, nblk, n_chunks


def _build_nc(n, npc, nblk, n_chunks, t4, gofs):
    t_total = int(t4.sum())
    f32 = mybir.dt.float32
    bf16 = mybir.dt.bfloat16
    i16 = mybir.dt.int16
    n_sb = math.ceil(nblk / SBN)

    nc = bacc.Bacc(
        "TRN2",
        target_bir_lowering=False,
        debug=False,
        enable_asserts=False,
        num_devices=NCORES,
    )

    t1 = nc.dram_tensor("t1", [n, IN_C], bf16, kind="ExternalInput").ap()
    srcs16 = nc.dram_tensor(
        "srcs16", [BLK, t_total * 8], i16, kind="ExternalInput"
    ).ap()
    dstf = nc.dram_tensor("dstf", [BLK, t_total], f32, kind="ExternalInput").ap()
    dinvB = nc.dram_tensor("dinvB", [BLK, nblk], f32, kind="ExternalInput").ap()
    dinvB2 = nc.dram_tensor("dinvB2", [BLK, nblk], f32, kind="ExternalInput").ap()
    biasB = nc.dram_tensor(
        "biasB", [BLK, nblk * OUT_C], f32, kind="ExternalInput"
    ).ap()
    iota = nc.dram_tensor("iota", [BLK, BLK], bf16, kind="ExternalInput").ap()
    W12 = nc.dram_tensor("W12", [IN_C, OUT_C], f32, kind="ExternalInput").ap()
    out = nc.dram_tensor("out", [npc, OUT_C], f32, kind="ExternalOutput").ap()

    ieq = mybir.AluOpType.is_equal
    mul = mybir.AluOpType.mult
    sub = mybir.AluOpType.subtract
    copyf = mybir.ActivationFunctionType.Copy

    def nb_of(b):
        return min(BLK, npc - b * BLK)

    def sb_blocks(sbi):
        return range(sbi * SBN, min((sbi + 1) * SBN, nblk))

    with tile.TileContext(nc) as tc:
        with (
            tc.tile_pool(name="dram", bufs=1, space="DRAM") as dram,
            tc.tile_pool(name="const", bufs=1) as constp,
            tc.tile_pool(name="meta", bufs=1) as metap,
            tc.tile_pool(name="msg", bufs=2) as msgp,
            tc.tile_pool(name="sel", bufs=6) as selp,
            tc.tile_pool(name="fin", bufs=3) as finp,
            tc.tile_pool(name="stat", bufs=4) as statp,
            tc.tile_pool(name="psA", bufs=6, space="PSUM") as psA,
            tc.tile_pool(name="psB", bufs=2, space="PSUM") as psB,
        ):
            g_shard = dram.tile([npc, IN_C], bf16)
            g_full = dram.tile([n, IN_C], bf16, addr_space="Shared")

            iotas = constp.tile([BLK, BLK], bf16)
            nc.sync.dma_start(out=iotas[:], in_=iota)
            W12s = constp.tile([IN_C, OUT_C], f32)
            nc.sync.dma_start(out=W12s[:], in_=W12)
            dinvBs = constp.tile([BLK, nblk], f32)
            nc.sync.dma_start(out=dinvBs[:], in_=dinvB)
            dinvB2s = constp.tile([BLK, nblk], f32)
            nc.sync.dma_start(out=dinvB2s[:], in_=dinvB2)
            biasBs = constp.tile([BLK, nblk * OUT_C], f32)
            nc.sync.dma_start(out=biasBs[:], in_=biasB)
            dstf_s = metap.tile([BLK, t_total], f32)
            nc.sync.dma_start(out=dstf_s[:], in_=dstf)
            srcs16_s = metap.tile([BLK, t_total * 8], i16)
            nc.sync.dma_start(out=srcs16_s[:], in_=srcs16)

            def gather_sb(sbi, table_full, elem, msg_tag):
                """One superblock's gathers: one call per chunk."""
                blocks = list(sb_blocks(sbi))
                sb_col0 = int(gofs[blocks[0], 0])
                t_sb = int(sum(t4[b, q] for b in blocks for q in range(n_chunks)))
                msg = msgp.tile([BLK, t_sb * elem], bf16, tag=msg_tag)
                for q in range(n_chunks):
                    cs = int(gofs[blocks[0], q])
                    tq = int(sum(t4[b, q] for b in blocks))
                    if tq == 0:
                        continue
                    hi_r = min((q + 1) * CHUNK_ROWS, n)
                    for k in range(0, tq, MAXT):
                        tk = min(MAXT, tq - k)
                        lo = cs - sb_col0 + k
                        nc.gpsimd.dma_gather(
                            out_ap=msg[:, lo * elem : (lo + tk) * elem].rearrange(
                                "p (t e) -> p t e", e=elem
                            ),
                            in_ap=table_full[q * CHUNK_ROWS : hi_r, :],
                            idxs_ap=srcs16_s[:, (cs + k) * 8 : (cs + k + tk) * 8],
                            num_idxs=tk * BLK,
                            num_idxs_reg=tk * BLK,
                            elem_size=elem,
                        )
                return msg, sb_col0

            def block_tiles(b):
                return [
                    int(gofs[b, q] + t)
                    for q in range(n_chunks)
                    for t in range(int(t4[b, q]))
                ]

            # ---- Phase A: g = dinv^2 * sum_e t1[src] per dst block ----
            for sbi in range(n_sb):
                msg, sb_col0 = gather_sb(sbi, t1, IN_C, "msg")
                for b in sb_blocks(sbi):
                    nb = nb_of(b)
                    cols = block_tiles(b)
                    ps = psA.tile([BLK, IN_C], f32, tag="psA")
                    for j, g in enumerate(cols):
                        S = selp.tile([BLK, BLK], bf16, tag="S")
                        nc.vector.tensor_scalar(
                            out=S[:],
                            in0=iotas[:],
                            scalar1=dstf_s[:, g : g + 1],
                            scalar2=None,
                            op0=ieq,
                        )
                        lo = g - sb_col0
                        nc.tensor.matmul(
                            out=ps[:nb, :],
                            lhsT=S[:, :nb],
                            rhs=msg[:, lo * IN_C : (lo + 1) * IN_C],
                            start=(j == 0),
                            stop=(j == len(cols) - 1),
                        )
                    gt = finp.tile([BLK, IN_C], bf16, tag="gt")
                    nc.scalar.activation(
                        gt[:nb, :], ps[:nb, :], copyf,
                        scale=dinvB2s[:nb, b : b + 1],
                    )
                    nc.sync.dma_start(
                        out=g_shard[b * BLK : b * BLK + nb, :], in_=gt[:nb, :]
                    )

            nc.gpsimd.collective_compute(
                "AllGather",
                mybir.AluOpType.bypass,
                replica_groups=[list(range(NCORES))],
                ins=[g_shard[:]],
                outs=[g_full[:]],
            )

            # ---- Phase B: z = dinv * (sum_e t2[src]) @ W12 + bias; normalize ----
            for sbi in range(n_sb):
                msg2, sb_col0 = gather_sb(sbi, g_full, IN_C, "msg")
                for b in sb_blocks(sbi):
                    nb = nb_of(b)
                    cols = block_tiles(b)
                    psT = psA.tile([IN_C, BLK], f32, tag="psA")
                    for j, g in enumerate(cols):
                        S = selp.tile([BLK, BLK], bf16, tag="S")
                        nc.vector.tensor_scalar(
                            out=S[:],
                            in0=iotas[:],
                            scalar1=dstf_s[:, g : g + 1],
                            scalar2=None,
                            op0=ieq,
                        )
                        lo = g - sb_col0
                        nc.tensor.matmul(
                            out=psT[:, :nb],
                            lhsT=msg2[:, lo * IN_C : (lo + 1) * IN_C],
                            rhs=S[:, :nb],
                            start=(j == 0),
                            stop=(j == len(cols) - 1),
                        )
                    a2 = finp.tile([IN_C, BLK], f32, tag="a2")
                    nc.scalar.copy(a2[:, :nb], psT[:, :nb])
                    zps = psB.tile([BLK, OUT_C], f32, tag="psB")
                    nc.tensor.matmul(
                        out=zps[:nb, :],
                        lhsT=a2[:, :nb],
                        rhs=W12s[:],
                        start=True,
                        stop=True,
                    )
                    z = finp.tile([BLK, OUT_C], f32, tag="z")
                    nc.scalar.activation(
                        z[:nb, :], zps[:nb, :], copyf,
                        scale=dinvBs[:nb, b : b + 1],
                    )
                    z2 = finp.tile([BLK, OUT_C], f32, tag="z2")
                    nc.vector.tensor_add(
                        z2[:nb, :], z[:nb, :],
                        biasBs[:nb, b * OUT_C : b * OUT_C + OUT_C],
                    )
                    zmax = statp.tile([BLK, 1], f32, tag="zmax")
                    nc.vector.tensor_reduce(
                        zmax[:nb], z2[:nb, :], axis=mybir.AxisListType.X,
                        op=mybir.AluOpType.max,
                    )
                    zmin = statp.tile([BLK, 1], f32, tag="zmin")
                    nc.vector.tensor_reduce(
                        zmin[:nb], z2[:nb, :], axis=mybir.AxisListType.X,
                        op=mybir.AluOpType.min,
                    )
                    rng_t = statp.tile([BLK, 1], f32, tag="rng")
                    nc.vector.tensor_sub(rng_t[:nb], zmax[:nb], zmin[:nb])
                    rinv = statp.tile([BLK, 1], f32, tag="rinv")
                    nc.vector.reciprocal(rinv[:nb], rng_t[:nb])
                    zs = finp.tile([BLK, OUT_C], f32, tag="zs")
                    nc.vector.tensor_scalar(
                        out=zs[:nb, :],
                        in0=z2[:nb, :],
                        scalar1=zmin[:nb],
                        scalar2=rinv[:nb],
                        op0=sub,
                        op1=mul,
                    )
                    sq = finp.tile([BLK, OUT_C], f32, tag="sq")
                    ssq = statp.tile([BLK, 1], f32, tag="ssq")
                    nc.scalar.activation(
                        sq[:nb, :],
                        zs[:nb, :],
                        mybir.ActivationFunctionType.Square,
                        accum_out=ssq[:nb],
                    )
                    snrm = statp.tile([BLK, 1], f32, tag="snrm")
                    nc.scalar.sqrt(snrm[:nb], ssq[:nb])
                    nc.vector.tensor_scalar_max(snrm[:nb], snrm[:nb], 1e-12)
                    ninv = statp.tile([BLK, 1], f32, tag="ninv")
                    nc.vector.reciprocal(ninv[:nb], snrm[:nb])
                    res = finp.tile([BLK, OUT_C], f32, tag="res")
                    nc.vector.tensor_scalar_mul(res[:nb, :], zs[:nb, :], ninv[:nb])
                    nc.sync.dma_start(
                        out=out[b * BLK : b * BLK + nb, :], in_=res[:nb, :]
                    )

    nc.compile()
    return nc


def kernel(x, edge_index, W1, b1, W2, b2, trace=False):
    global LAST_RESULTS
    x = np.asarray(x)
    edge_index = np.asarray(edge_index)
    W1 = np.asarray(W1, dtype=np.float32)
    b1 = np.asarray(b1, dtype=np.float32)
    W2 = np.asarray(W2, dtype=np.float32)
    b2 = np.asarray(b2, dtype=np.float32)

    n = x.shape[0]
    in_maps, t4, gofs, npc, nblk, n_chunks = _host_prep(
        x, edge_index, W1, b1, W2, b2
    )

    key = (n, t4.tobytes())
    nc = _PROGRAM_CACHE.get(key)
    if nc is None:
        nc = _build_nc(n, npc, nblk, n_chunks, t4, gofs)
        _PROGRAM_CACHE[key] = nc

    results = bass_utils.run_bass_kernel_spmd(
        nc, in_maps, core_ids=list(range(NCORES)), trace=trace
    )
    LAST_RESULTS = results
    return np.concatenate([results.results[c]["out"] for c in range(NCORES)], axis=0)


# revision 11
# speedup vs baseline: 2.4392x; 2.3559x over previous
"""Two-layer GCN encoder (GCNConv x2 + minmax + L2 normalize) on 8 TRN2 NeuronCores.

Algebra: with A = D^-1/2 (Adj+I) D^-1/2 and no nonlinearity between the two
GCNConv layers, out = minmax_l2( A.(A.x).(W1@W2) + rowsumA.(b1@W2) + b2 ).
The symmetric norm factorizes: each aggregation is dinv[d] * sum_e t[src_e]
with the table pre-scaled by dinv (t1 = dinv*x on host, t2 = dinv^2*(A-sum)
on device), so the per-tile selection matrix S[e,j] = (dst_local[e]==j) is
0/1 and built with a single is_equal DVE op in bf16.

Sharding: nodes row-partitioned across 8 cores (12500/core); each edge owned
by the core owning its destination. Edges grouped by 128-node destination
block and 25000-row source chunk (dma_gather int16 indices), padded to
128-edge tiles; per-(block,chunk) tile counts are equalized across cores so
one SPMD program serves all 8. Gathers issue one dma_gather per (superblock
of 4 blocks, chunk) (~23 tiles = ~2900 indices per call); trailing pad slots
carry -1 indices which the Q7 gather kernel trims at runtime.

Phase A aggregates t1 node-major per block (psum += S^T @ Msg), scales by
dinv^2 on ScalarE, stores bf16 shard, AllGathers to the full t2 table.
Phase B aggregates t2 transposed (psum += Msg^T @ S), applies W12 via a
second matmul, then dinv scale (ScalarE) + bias + minmax + L2 normalize.
"""

import math

import numpy as np
import ml_dtypes

import concourse.bass as bass
import concourse.bacc as bacc
import concourse.mybir as mybir
import concourse.tile as tile
from concourse import bass_utils

NCORES = 8
BLK = 128
IN_C = 128
HID = 128
OUT_C = 64
CHUNK_ROWS = 25000  # dma_gather idx is int16: chunk-relative indices < 32768
SBN = 4  # destination blocks per gather superblock
MAXT = 8  # tiles per dma_gather call (1024 idx, 65 ring descs/lane: known-safe)
NQ = 4    # rotate dma_gather queue_num to decouple descgen from drain

BF16 = ml_dtypes.bfloat16

LAST_RESULTS = None
_PROGRAM_CACHE = {}


def _host_prep(x, edge_index, W1, b1, W2, b2):
    n = x.shape[0]
    assert n % NCORES == 0
    npc = n // NCORES
    nblk = math.ceil(npc / BLK)
    n_chunks = math.ceil(n / CHUNK_ROWS)

    src = edge_index[0].astype(np.int64)
    dst = edge_index[1].astype(np.int64)

    deg = (np.bincount(dst, minlength=n) + 1).astype(np.float32)
    dinv = (1.0 / np.sqrt(deg)).astype(np.float32)

    loop = np.arange(n, dtype=np.int64)
    s_all = np.concatenate([src, loop])
    d_all = np.concatenate([dst, loop])

    # rowsumA[d] = dinv[d] * sum_{e->d} dinv[src_e]  (self-loop included)
    acc = np.zeros(n, np.float32)
    np.add.at(acc, d_all, dinv[s_all])
    rowsumA = dinv * acc

    W12 = (W1.astype(np.float64) @ W2.astype(np.float64)).astype(np.float32)
    b1W2 = (b1 @ W2).astype(np.float32)
    bias_full = (rowsumA[:, None] * b1W2[None, :] + b2[None, :]).astype(np.float32)

    t1 = (x * dinv[:, None]).astype(BF16)

    core = d_all // npc
    within = d_all % npc
    blk = within // BLK
    colv = (within % BLK).astype(np.float32)
    chunk = s_all // CHUNK_ROWS

    key = (core * nblk + blk) * n_chunks + chunk
    counts = np.bincount(key, minlength=NCORES * nblk * n_chunks).reshape(
        NCORES, nblk * n_chunks
    )
    # tiles per (block, chunk), equalized across cores (SPMD)
    t4 = ((counts + BLK - 1) // BLK).max(axis=0).reshape(nblk, n_chunks)

    # global tile order: for each superblock: for each chunk: for each block
    gofs = np.zeros((nblk, n_chunks), np.int64)
    cur = 0
    n_sb = math.ceil(nblk / SBN)
    for sbi in range(n_sb):
        for q in range(n_chunks):
            for b in range(sbi * SBN, min((sbi + 1) * SBN, nblk)):
                gofs[b, q] = cur
                cur += int(t4[b, q])
    t_total = cur

    order = np.argsort(key, kind="stable")
    ks = key[order]
    ss = s_all[order]
    cs = colv[order]

    group_start = np.zeros(NCORES * nblk * n_chunks, np.int64)
    group_start[1:] = np.cumsum(counts.ravel())[:-1]
    r = np.arange(len(ks), dtype=np.int64) - group_start[ks]
    t_idx = r // BLK
    p_idx = r % BLK
    c_idx = ks // (nblk * n_chunks)
    b_idx = (ks // n_chunks) % nblk
    q_idx = ks % n_chunks
    gcol = gofs[b_idx, q_idx] + t_idx
    rel = (ss - q_idx * CHUNK_ROWS).astype(np.int16)

    # int16 idx stream for dma_gather: index k of a call lives at
    # [k%16 (+16*replica), call_col0*8 + k//16]; with 128-multiple groups this
    # reduces to [p%16, gcol*8 + p//16] independent of call boundaries.
    srcs16 = np.zeros((NCORES, 16, t_total * 8), np.int16)
    dstf_arr = np.full((NCORES, BLK, t_total), -1.0, np.float32)
    srcs16[c_idx, p_idx % 16, gcol * 8 + p_idx // 16] = rel
    dstf_arr[c_idx, p_idx, gcol] = cs

    # Pad slots keep idx 0 (safe in-bounds read; their dstf=-1 zeroes S).
    # NOTE: -1 trailing-trim is unusable under SPMD — the Q7 kernel trims by
    # value but the sequencer reserves ring space by num_idxs_reg, which is a
    # compile-time constant shared across cores; a mismatch drifts the ring
    # offsets against the SDMA tail and executes stale descriptors.
    srcs16 = np.tile(srcs16, (1, 8, 1))  # replicate for the 8 Q7 cores

    # per-node dinv / dinv^2 by (block-local row, block); bias by block
    pad_npc = nblk * BLK
    dinv_pad = np.zeros((NCORES, pad_npc), np.float32)
    dinv_pad[:, :npc] = dinv.reshape(NCORES, npc)
    dinvB = np.ascontiguousarray(
        dinv_pad.reshape(NCORES, nblk, BLK).transpose(0, 2, 1)
    )
    bias_pad = np.zeros((NCORES, pad_npc, OUT_C), np.float32)
    bias_pad[:, :npc] = bias_full.reshape(NCORES, npc, OUT_C)
    biasB = np.ascontiguousarray(
        bias_pad.reshape(NCORES, nblk, BLK, OUT_C).transpose(0, 2, 1, 3)
    ).reshape(NCORES, BLK, nblk * OUT_C)

    iota = np.tile(np.arange(BLK, dtype=np.float32), (BLK, 1)).astype(BF16)

    in_maps = []
    for c in range(NCORES):
        in_maps.append(
            {
                "t1": t1,
                "srcs16": np.ascontiguousarray(srcs16[c]),
                "dstf": np.ascontiguousarray(dstf_arr[c]),
                "dinvB": np.ascontiguousarray(dinvB[c]),
                "dinvB2": np.ascontiguousarray(dinvB[c] ** 2),
                "biasB": np.ascontiguousarray(biasB[c]),
                "iota": iota,
                "W12": np.ascontiguousarray(W12),
            }
        )
    return in_maps, t4, gofs, npc, nblk, n_chunks


def _build_nc(n, npc, nblk, n_chunks, t4, gofs):
    t_total = int(t4.sum())
    f32 = mybir.dt.float32
    bf16 = mybir.dt.bfloat16
    i16 = mybir.dt.int16
    n_sb = math.ceil(nblk / SBN)

    nc = bacc.Bacc(
        "TRN2",
        target_bir_lowering=False,
        debug=False,
        enable_asserts=False,
        num_devices=NCORES,
        num_swdge_queues=NQ,
    )

    t1 = nc.dram_tensor("t1", [n, IN_C], bf16, kind="ExternalInput").ap()
    srcs16 = nc.dram_tensor(
        "srcs16", [BLK, t_total * 8], i16, kind="ExternalInput"
    ).ap()
    dstf = nc.dram_tensor("dstf", [BLK, t_total], f32, kind="ExternalInput").ap()
    dinvB = nc.dram_tensor("dinvB", [BLK, nblk], f32, kind="ExternalInput").ap()
    dinvB2 = nc.dram_tensor("dinvB2", [BLK, nblk], f32, kind="ExternalInput").ap()
    biasB = nc.dram_tensor(
        "biasB", [BLK, nblk * OUT_C], f32, kind="ExternalInput"
    ).ap()
    iota = nc.dram_tensor("iota", [BLK, BLK], bf16, kind="ExternalInput").ap()
    W12 = nc.dram_tensor("W12", [IN_C, OUT_C], f32, kind="ExternalInput").ap()
    out = nc.dram_tensor("out", [npc, OUT_C], f32, kind="ExternalOutput").ap()

    ieq = mybir.AluOpType.is_equal
    mul = mybir.AluOpType.mult
    sub = mybir.AluOpType.subtract
    copyf = mybir.ActivationFunctionType.Copy

    def nb_of(b):
        return min(BLK, npc - b * BLK)

    def sb_blocks(sbi):
        return range(sbi * SBN, min((sbi + 1) * SBN, nblk))

    with tile.TileContext(nc) as tc:
        with (
            tc.tile_pool(name="dram", bufs=1, space="DRAM") as dram,
            tc.tile_pool(name="const", bufs=1) as constp,
            tc.tile_pool(name="meta", bufs=1) as metap,
            tc.tile_pool(name="msg", bufs=2) as msgp,
            tc.tile_pool(name="sel", bufs=6) as selp,
            tc.tile_pool(name="fin", bufs=3) as finp,
            tc.tile_pool(name="stat", bufs=4) as statp,
            tc.tile_pool(name="psA", bufs=6, space="PSUM") as psA,
            tc.tile_pool(name="psB", bufs=2, space="PSUM") as psB,
        ):
            g_shard = dram.tile([npc, IN_C], bf16)
            g_full = dram.tile([n, IN_C], bf16, addr_space="Shared")

            iotas = constp.tile([BLK, BLK], bf16)
            nc.sync.dma_start(out=iotas[:], in_=iota)
            W12s = constp.tile([IN_C, OUT_C], f32)
            nc.sync.dma_start(out=W12s[:], in_=W12)
            dinvBs = constp.tile([BLK, nblk], f32)
            nc.sync.dma_start(out=dinvBs[:], in_=dinvB)
            dinvB2s = constp.tile([BLK, nblk], f32)
            nc.sync.dma_start(out=dinvB2s[:], in_=dinvB2)
            biasBs = constp.tile([BLK, nblk * OUT_C], f32)
            nc.sync.dma_start(out=biasBs[:], in_=biasB)
            dstf_s = metap.tile([BLK, t_total], f32)
            nc.sync.dma_start(out=dstf_s[:], in_=dstf)
            srcs16_s = metap.tile([BLK, t_total * 8], i16)
            nc.sync.dma_start(out=srcs16_s[:], in_=srcs16)

            qrot = [0]

            def gather_sb(sbi, table_full, elem, msg_tag):
                """One superblock's gathers, queue-rotated across calls."""
                blocks = list(sb_blocks(sbi))
                sb_col0 = int(gofs[blocks[0], 0])
                t_sb = int(sum(t4[b, q] for b in blocks for q in range(n_chunks)))
                msg = msgp.tile([BLK, t_sb * elem], bf16, tag=msg_tag)
                for q in range(n_chunks):
                    cs = int(gofs[blocks[0], q])
                    tq = int(sum(t4[b, q] for b in blocks))
                    if tq == 0:
                        continue
                    hi_r = min((q + 1) * CHUNK_ROWS, n)
                    for k in range(0, tq, MAXT):
                        tk = min(MAXT, tq - k)
                        lo = cs - sb_col0 + k
                        nc.gpsimd.dma_gather(
                            out_ap=msg[:, lo * elem : (lo + tk) * elem].rearrange(
                                "p (t e) -> p t e", e=elem
                            ),
                            in_ap=table_full[q * CHUNK_ROWS : hi_r, :],
                            idxs_ap=srcs16_s[:, (cs + k) * 8 : (cs + k + tk) * 8],
                            num_idxs=tk * BLK,
                            num_idxs_reg=tk * BLK,
                            elem_size=elem,
                            queue_num=qrot[0],
                        )
                        qrot[0] = (qrot[0] + 1) % NQ
                return msg, sb_col0

            def block_tiles(b):
                return [
                    int(gofs[b, q] + t)
                    for q in range(n_chunks)
                    for t in range(int(t4[b, q]))
                ]

            # ---- Phase A: g = dinv^2 * sum_e t1[src] per dst block ----
            for sbi in range(n_sb):
                msg, sb_col0 = gather_sb(sbi, t1, IN_C, "msg")
                for b in sb_blocks(sbi):
                    nb = nb_of(b)
                    cols = block_tiles(b)
                    ps = psA.tile([BLK, IN_C], f32, tag="psA")
                    for j, g in enumerate(cols):
                        S = selp.tile([BLK, BLK], bf16, tag="S")
                        nc.vector.tensor_scalar(
                            out=S[:],
                            in0=iotas[:],
                            scalar1=dstf_s[:, g : g + 1],
                            scalar2=None,
                            op0=ieq,
                        )
                        lo = g - sb_col0
                        nc.tensor.matmul(
                            out=ps[:nb, :],
                            lhsT=S[:, :nb],
                            rhs=msg[:, lo * IN_C : (lo + 1) * IN_C],
                            start=(j == 0),
                            stop=(j == len(cols) - 1),
                        )
                    gt = finp.tile([BLK, IN_C], bf16, tag="gt")
                    nc.scalar.activation(
                        gt[:nb, :], ps[:nb, :], copyf,
                        scale=dinvB2s[:nb, b : b + 1],
                    )
                    nc.sync.dma_start(
                        out=g_shard[b * BLK : b * BLK + nb, :], in_=gt[:nb, :]
                    )

            nc.gpsimd.collective_compute(
                "AllGather",
                mybir.AluOpType.bypass,
                replica_groups=[list(range(NCORES))],
                ins=[g_shard[:]],
                outs=[g_full[:]],
            )

            # ---- Phase B: z = dinv * (sum_e t2[src]) @ W12 + bias; normalize ----
            for sbi in range(n_sb):
                msg2, sb_col0 = gather_sb(sbi, g_full, IN_C, "msg")
                for b in sb_blocks(sbi):
                    nb = nb_of(b)
                    cols = block_tiles(b)
                    psT = psA.tile([IN_C, BLK], f32, tag="psA")
                    for j, g in enumerate(cols):
                        S = selp.tile([BLK, BLK], bf16, tag="S")
                        nc.vector.tensor_scalar(
                            out=S[:],
                            in0=iotas[:],
                            scalar1=dstf_s[:, g : g + 1],
                            scalar2=None,
                            op0=ieq,
                        )
                        lo = g - sb_col0
                        nc.tensor.matmul(
                            out=psT[:, :nb],
                            lhsT=msg2[:, lo * IN_C : (lo + 1) * IN_C],
                            rhs=S[:, :nb],
                            start=(j == 0),
                            stop=(j == len(cols) - 1),
                        )
                    a2 = finp.tile([IN_C, BLK], f32, tag="a2")
                    nc.scalar.copy(a2[:, :nb], psT[:, :nb])
                    zps = psB.tile([BLK, OUT_C], f32, tag="psB")
                    nc.tensor.matmul(
                        out=zps[:nb, :],
                        lhsT=a2[:, :nb],
                        rhs=W12s[:],
                        start=True,
                        stop=True,
                    )
                    z = finp.tile([BLK, OUT_C], f32, tag="z")
                    nc.scalar.activation(
                        z[:nb, :], zps[:nb, :], copyf,
                        scale=dinvBs[:nb, b : b + 1],
                    )
                    z2 = finp.tile([BLK, OUT_C], f32, tag="z2")
                    nc.vector.tensor_add(
                        z2[:nb, :], z[:nb, :],
                        biasBs[:nb, b * OUT_C : b * OUT_C + OUT_C],
                    )
                    zmax = statp.tile([BLK, 1], f32, tag="zmax")
                    nc.vector.tensor_reduce(
                        zmax[:nb], z2[:nb, :], axis=mybir.AxisListType.X,
                        op=mybir.AluOpType.max,
                    )
                    zmin = statp.tile([BLK, 1], f32, tag="zmin")
                    nc.vector.tensor_reduce(
                        zmin[:nb], z2[:nb, :], axis=mybir.AxisListType.X,
                        op=mybir.AluOpType.min,
                    )
                    rng_t = statp.tile([BLK, 1], f32, tag="rng")
                    nc.vector.tensor_sub(rng_t[:nb], zmax[:nb], zmin[:nb])
                    rinv = statp.tile([BLK, 1], f32, tag="rinv")
                    nc.vector.reciprocal(rinv[:nb], rng_t[:nb])
                    zs = finp.tile([BLK, OUT_C], f32, tag="zs")
                    nc.vector.tensor_scalar(
                        out=zs[:nb, :],
                        in0=z2[:nb, :],
                        scalar1=zmin[:nb],
                        scalar2=rinv[:nb],
                        op0=sub,
                        op1=mul,
                    )
                    sq = finp.tile([BLK, OUT_C], f32, tag="sq")
                    ssq = statp.tile([BLK, 1], f32, tag="ssq")
                    nc.scalar.activation(
                        sq[:nb, :],
                        zs[:nb, :],
                        mybir.ActivationFunctionType.Square,
                        accum_out=ssq[:nb],
                    )
                    snrm = statp.tile([BLK, 1], f32, tag="snrm")
                    nc.scalar.sqrt(snrm[:nb], ssq[:nb])
                    nc.vector.tensor_scalar_max(snrm[:nb], snrm[:nb], 1e-12)
                    ninv = statp.tile([BLK, 1], f32, tag="ninv")
                    nc.vector.reciprocal(ninv[:nb], snrm[:nb])
                    res = finp.tile([BLK, OUT_C], f32, tag="res")
                    nc.vector.tensor_scalar_mul(res[:nb, :], zs[:nb, :], ninv[:nb])
                    nc.sync.dma_start(
                        out=out[b * BLK : b * BLK + nb, :], in_=res[:nb, :]
                    )

    nc.compile()
    return nc


def kernel(x, edge_index, W1, b1, W2, b2, trace=False):
    global LAST_RESULTS
    x = np.asarray(x)
    edge_index = np.asarray(edge_index)
    W1 = np.asarray(W1, dtype=np.float32)
    b1 = np.asarray(b1, dtype=np.float32)
    W2 = np.asarray(W2, dtype=np.float32)
    b2 = np.asarray(b2, dtype=np.float32)

    n = x.shape[0]
    in_maps, t4, gofs, npc, nblk, n_chunks = _host_prep(
        x, edge_index, W1, b1, W2, b2
    )

    key = (n, t4.tobytes())
    nc = _PROGRAM_CACHE.get(key)
    if nc is None:
        nc = _build_nc(n, npc, nblk, n_chunks, t4, gofs)
        _PROGRAM_CACHE[key] = nc

    results = bass_utils.run_bass_kernel_spmd(
        nc, in_maps, core_ids=list(range(NCORES)), trace=trace
    )
    LAST_RESULTS = results
    return np.concatenate([results.results[c]["out"] for c in range(NCORES)], axis=0)
